# revision 31
# baseline (speedup 1.0000x reference)
"""Windowed attention + dynamic relative position bias on 8 NeuronCores.

Shapes: q,k,v [B=128, H=8, N=256, D=32] f32; pos-MLP width P=16; h=w=16.
Sharding: head-parallel - core c computes head c for all 128 batch windows;
the per-core head is selected purely by the bias tables passed to that core
(program is SPMD-identical).

Design (multi-engine exp, fp16 IO, host normalization):
  - All math in "z units": S = alpha*qk with alpha = 1024*log2(e)/sqrt(D);
    exp(x) == 2^(Z/1024) for Z in z units.
  - The tiny dynamic-pos-bias MLP (0.005% of the FLOPs) and its Toeplitz
    gather are computed on the host (like the other host-packed constants)
    and shipped per-core as two tables in z units:
      bfold [128,(mb,n)] f32r : 1024*log2e*bias - 6144, fold-ready
      expbd [128,(mb,n)] fp16 : 2^(bias - 6), for post-exp multiplies
  - Softmax exp is split across TWO engines per tile ([128,1024] of S):
      Act: activation Exp (scale=ln2/1024) -> fp16 E
      DVE: custom fused op EXP2_BITS_ANT computing the BITS of fp16(2^z):
           Z=S+zkf; N=(Z+B)-B (magic round to 1024s); F=Z-N;
           u16=trunc(C2*F*F + Z + C1).  One DVE instr per tile; max rel
           err ~0.2%.  Src1 must be constant (zkf); the bias is PE-folded.
  - Bias application per tile: PE fold (identity-matmul accumulate of
    bfold into S; used by all DVE tiles + F-quota Act tiles) or post-exp
    fp16 multiply by expbd (on DVE or Pool).
  - No on-device normalization: PV uses ones-augmented V; raw O and Z
    columns are evacuated PSUM->SBUF (Act Copy / DVE copy; Copy shares
    the Exp act table so no table reloads), DMA'd out as fp16, and the
    host divides O/Z.  Per-tile exp scale constants differ by path but
    every softmax row lives inside one tile, so they cancel in the
    division.
  - q/k/v host-packed fp16 (q pre-scaled by alpha): halves DMA bytes.
"""

import os
import numpy as np

B, H, N, D = 128, 8, 256, 32
NCORES = 8
NGROUPS = 16
CHUNKS = [(0, 1), (2, 5), (6, 10), (11, 15)]
CHUNK_OF_GROUP = [0, 0, 1, 1, 1, 1, 2, 2, 2, 2, 2, 3, 3, 3, 3, 3]

LOG2E = 1.4426950408889634
ALPHA = float(1024.0 * LOG2E / np.sqrt(D))         # q prescale (host)
ACT_SCALE = float(np.log(2.0) / 1024.0)            # Act exp scale in z units
W3_SCALE = float(1024.0 * LOG2E)                   # bias table scale
BSHIFT = -6144.0                                   # -6 octaves, inside bfold
BMAGIC = float(1.5 * 2 ** 33)                      # fp32 round-to-1024 magic
EXP_C1 = 433.57                                    # mantissa-parabola const
EXP_C2 = 3.3007e-4                                 # mantissa-parabola curv
ZKF_CONST = 12800.0 + 6144.0                       # folded-DVE zbias const

# tile modes: 'A' = Act exp + emul; 'F' = Act exp + PE fold;
#             'D' = DVE custom exp + PE fold.
X_CNT = int(os.environ.get("K_X", "21"))           # DVE-exp tiles (of 64)
F_CNT = int(os.environ.get("K_F", "11"))            # PE-fold Act tiles
MDVE_CNT = int(os.environ.get("K_MDVE", "22"))     # DVE-emul quota (rest Pool)
EVA_ACT = int(os.environ.get("K_EVA", "10"))        # PSUM evacs on Act (of 32)

_BUILD_CACHE = {}


def _tile_modes():
    """Assign A/F/D to each of the 64 tiles, plus emul engine for A."""
    modes = [None] * 64
    acc_d = acc_f = 0
    for i in range(64):
        nd = ((i + 1) * X_CNT) // 64
        if nd > acc_d:
            modes[i] = "D"
            acc_d = nd
        else:
            nf = ((i + 1) * F_CNT) // 64
            if nf > acc_f:
                modes[i] = "F"
                acc_f = nf
            else:
                modes[i] = "A"
    for t in (62, 63):
        if modes[t] == "A":
            modes[t] = "F"
    # very first tile on Act (fastest start)
    if modes[0] != "A":
        swap = next(t for t in range(1, 64) if modes[t] == "A")
        modes[swap] = modes[0]
        modes[0] = "A"
    eng = {}
    need = [t for t in range(64) if modes[t] == "A"]
    nm = max(len(need), 1)
    for j, t in enumerate(need):
        if ((j + 1) * MDVE_CNT) // nm > (j * MDVE_CNT) // nm:
            eng[t] = "dve"
        else:
            eng[t] = "pool"
    return modes, eng


def _register_exp_op():
    if "op" in _BUILD_CACHE:
        return _BUILD_CACHE["op"]
    from concourse.dve_spec import Spec, Src0, Src1, C0, C1, C2, lower
    from concourse import dve_ops
    from concourse.dve_table_gen import dve_ver_for
    from concourse.dve_uop import DveOpSpec

    for o in dve_ops.OPS:
        if o.name == "EXP2_BITS_ANT":
            _BUILD_CACHE["op"] = o
            return o

    Z = Src0 + Src1
    Nq = (Z + C0) - C0
    F = Z - Nq
    body = (C2 * F) * F + (Z + C1)

    def ref(in0, in1, s0, s1, imm2):
        f32 = np.float32
        Zv = f32(f32(in0.astype(np.float32)) + f32(in1.astype(np.float32)))
        t = f32(Zv + f32(s0))
        Nv = f32(t - f32(s0))
        Fv = f32(Zv - Nv)
        u = f32(f32(f32(f32(imm2)) * Fv) * Fv + f32(Zv + f32(s1)))
        return np.clip(u, 0.0, 65535.0)

    spec = Spec(body=body, reference=ref)
    ver = dve_ver_for("TRN2")
    row = dve_ops._CUSTOM_DVE_ROW_BASE + len(dve_ops.OPS)
    sha = DveOpSpec(name="EXP2_BITS_ANT", opcode=row,
                    uops=lower(spec, ver=ver), rd1_en=True).sha(ver)
    op = dve_ops.DveOp("EXP2_BITS_ANT", spec, subdim=False,
                       uops_sha={ver: sha})
    dve_ops.OPS.append(op)
    dve_ops.CUSTOM_DVE_SPECS[op.name] = spec
    dve_ops._SUB_OPCODE_FOR_NAME[op.name] = row
    _BUILD_CACHE["op"] = op
    return op


def _build():
    if "nc" in _BUILD_CACHE:
        return _BUILD_CACHE["nc"]
    import concourse.bacc as bacc
    import concourse.mybir as mybir
    from concourse.tile import TileContext
    from bass_rust import AP

    exp_op = _register_exp_op()

    F32 = mybir.dt.float32
    F32R = mybir.dt.float32r
    FP16 = mybir.dt.float16
    U16 = mybir.dt.uint16
    AF = mybir.ActivationFunctionType

    nc = bacc.Bacc("TRN2", target_bir_lowering=False, debug=False,
                   num_devices=NCORES)

    # host-prearranged layouts (see build_in_maps), all fp16:
    # qd [128 p=(bi,d), (g 16, hh 2, n 256)] fp16, pre-scaled by ALPHA
    # kd [128 p=(bi,d), (g 16, hh 2, mb 2, m 128)] fp16
    # vd [128 p=m, (b 128, c 2, e 33)] fp16 (e==32 -> 1.0)
    qd = nc.dram_tensor("qd", [128, 8192], FP16, kind="ExternalInput")
    kd = nc.dram_tensor("kd", [128, 8192], FP16, kind="ExternalInput")
    vd = nc.dram_tensor("vd", [128, 8448], FP16, kind="ExternalInput")
    bfold_d = nc.dram_tensor("bfold", [128, 512], F32R, kind="ExternalInput")
    expb_d = nc.dram_tensor("expbd", [128, 512], FP16, kind="ExternalInput")
    ident_d = nc.dram_tensor("identd", [128, 128], F32, kind="ExternalInput")

    # raw O (32 cols) + Z (1 col) per j, 8 j per half-group, fp16
    out_d = nc.dram_tensor("out", [128, 8448], FP16, kind="ExternalOutput")

    MODES, EMUL_ENG = _tile_modes()

    with TileContext(nc) as tc:
        with (
            tc.tile_pool(name="const", bufs=1) as constp,
            tc.tile_pool(name="vpool", bufs=1) as vpool,
            tc.tile_pool(name="epool", bufs=int(os.environ.get("K_EP", "28"))) as epool,
            tc.tile_pool(name="spsum", bufs=int(os.environ.get("K_SB", "3")), space="PSUM") as spsum,
            tc.tile_pool(name="auxpsum", bufs=int(os.environ.get("K_AB", "2")), space="PSUM") as auxpsum,
        ):
            # ---- full-size q/k/v SBUF tiles; chunked loads emitted lazily
            q_all = vpool.tile([128, 8192], FP16)
            k_all = vpool.tile([128, 8192], FP16)
            v_all = vpool.tile([128, 8448], FP16)

            chunk_loaded = [False] * len(CHUNKS)

            def emit_chunk(ci):
                g0, g1 = CHUNKS[ci]
                ng = g1 - g0 + 1
                qk0 = g0
                if ci == 0:
                    qk0 = 1      # group 0 of q/k loaded via the fast path
                # q on Pool SWDGE; k/v on sync (HWDGE)
                nc.gpsimd.dma_start(
                    q_all[:, 512 * qk0:512 * (g1 + 1)],
                    AP(qd, 512 * qk0,
                       [[8192, 128], [1, 512 * (g1 - qk0 + 1)]]))
                nc.sync.dma_start(
                    k_all[:, 512 * qk0:512 * (g1 + 1)],
                    AP(kd, 512 * qk0,
                       [[8192, 128], [1, 512 * (g1 - qk0 + 1)]]))
                nc.sync.dma_start(
                    v_all[:, 528 * g0:528 * (g1 + 1)],
                    AP(vd, 528 * g0, [[8448, 128], [1, 528 * ng]]))

            def ensure_chunk(ci):
                if not chunk_loaded[ci]:
                    chunk_loaded[ci] = True
                    emit_chunk(ci)

            # fast path for the very first QK: k via sync HWDGE, q via Pool
            # SWDGE - different dispatchers run concurrently
            nc.sync.dma_start(k_all[:, 0:256],
                              AP(kd, 0, [[8192, 128], [1, 256]]))
            nc.gpsimd.dma_start(q_all[:, 0:256],
                                AP(qd, 0, [[8192, 128], [1, 256]]))
            nc.sync.dma_start(k_all[:, 256:512],
                              AP(kd, 256, [[8192, 128], [1, 256]]))
            nc.gpsimd.dma_start(q_all[:, 256:512],
                                AP(qd, 256, [[8192, 128], [1, 256]]))

            # bias tables + fold identity (scalar queue: its seq is free
            # early and the transfers are small)
            zkf = constp.tile([128, 1024], F32)
            nc.gpsimd.memset(zkf[:, :], ZKF_CONST)
            bfold = constp.tile([128, 512], F32R)
            nc.scalar.dma_start(bfold[:, :], bfold_d[:, :])
            expb = constp.tile([128, 512], FP16)
            nc.scalar.dma_start(expb[:, :], expb_d[:, :])
            ident_f = constp.tile([128, 128], F32)
            nc.scalar.dma_start(ident_f[:, :], ident_d[:, :])
            ident_r = constp.tile([128, 128], F32R)
            nc.vector.tensor_copy(ident_r[:, :], ident_f[:, :])

            ensure_chunk(0)
            ensure_chunk(1)

            # --- main pipeline over 64 tiles (2 per half-group) ---
            def emit_qk_exp(g, hh, split_exp=False):
                """QK matmuls (+bias folds) + exp -> epair, pending emuls."""
                ho = 512 * g + 256 * hh
                epair = []
                emuls = []
                for half in range(2):
                    t_idx = 4 * g + 2 * hh + half
                    mode = MODES[t_idx]
                    fold = mode != "A"
                    sp = spsum.tile([128, 1024], F32, tag="S",
                                    name=f"s{g}_{hh}_{half}")
                    for bi2 in range(2):
                        bi = 2 * half + bi2
                        fo = 512 * bi2
                        for mb in range(2):
                            out_ap = sp[:, fo + 256 * mb:fo + 256 * mb + 256]
                            nc.tensor.matmul(
                                out_ap,
                                k_all[32 * bi:32 * bi + 32,
                                      ho + 128 * mb:ho + 128 * mb + 128],
                                q_all[32 * bi:32 * bi + 32, ho:ho + 256],
                                tile_position=(32 * bi, 0),
                                start=True, stop=not fold)
                            if fold:
                                nc.tensor.matmul(
                                    out_ap, ident_r[:, :],
                                    bfold[:, 256 * mb:256 * mb + 256],
                                    tile_position=(0, 0),
                                    start=False, stop=True)
                    e = epool.tile([128, 1024], FP16, tag="E",
                                   name=f"e{g}_{hh}_{half}")
                    if mode == "D":
                        nc.vector._custom_dve(
                            exp_op, out=e[:, :].bitcast(U16),
                            in0=sp[:, :], in1=zkf[:, :],
                            s0=BMAGIC, s1=EXP_C1, imm2=EXP_C2)
                    else:
                        if split_exp:
                            nc.scalar.activation(e[:, :512], sp[:, :512],
                                                 AF.Exp, scale=ACT_SCALE)
                            nc.scalar.activation(e[:, 512:], sp[:, 512:],
                                                 AF.Exp, scale=ACT_SCALE)
                        else:
                            nc.scalar.activation(e[:, :], sp[:, :], AF.Exp,
                                                 scale=ACT_SCALE)
                        if mode == "A":
                            emuls.append((half, EMUL_ENG[t_idx]))
                    epair.append(e)
                return epair, emuls

            def emit_emul(epair, emuls):
                for half, eng in emuls:
                    e = epair[half]
                    e4 = e[:, :].rearrange("p (j mb n) -> p j mb n",
                                           mb=2, n=256)
                    bb = (expb[:, :].rearrange("p (mb n) -> p mb n", n=256)
                          .unsqueeze(1).broadcast_to((128, 2, 2, 256)))
                    if eng == "pool":
                        nc.gpsimd.tensor_mul(e4, e4, bb)
                    else:
                        nc.vector.tensor_mul(e4, e4, bb)

            def emit_pv_half(g, hh, epair, half, o_ps=None):
                if o_ps is None:
                    o_ps = auxpsum.tile([128, 264], F32, tag="aux2",
                                        name=f"ops{g}_{hh}")
                e = epair[half]
                for bi2 in range(2):
                    bi = 2 * half + bi2
                    fo = 512 * bi2
                    vb = 66 * (8 * g + 4 * hh + bi)
                    for nb in range(2):
                        j = 2 * bi + nb
                        for c in range(2):
                            nc.tensor.matmul(
                                o_ps[:, 33 * j:33 * j + 33],
                                e[:, fo + 256 * c + 128 * nb:
                                  fo + 256 * c + 128 * nb + 128],
                                v_all[:, vb + 33 * c:vb + 33 * c + 33],
                                start=(c == 0), stop=(c == 1))
                return o_ps

            evac_ctr = [0]

            def emit_evac_store(g, hh, o_ps, split=False):
                osb = epool.tile([128, 264], FP16, tag="osb",
                                 name=f"osb{g}_{hh}")

                def one(j0, nj):
                    src_ = o_ps[:, 33 * j0:33 * (j0 + nj)]
                    dst_ = osb[:, 33 * j0:33 * (j0 + nj)]
                    i = evac_ctr[0]
                    evac_ctr[0] += 1
                    if ((i + 1) * EVA_ACT) // 32 > (i * EVA_ACT) // 32:
                        nc.scalar.activation(dst_, src_, AF.Copy)
                    else:
                        nc.vector.tensor_copy(dst_, src_)
                    nc.sync.dma_start(
                        AP(out_d, 264 * (2 * g + hh) + 33 * j0,
                           [[8448, 128], [1, 33 * nj]]),
                        osb[:, 33 * j0:33 * (j0 + nj)])

                if split:
                    one(0, 4)
                    one(4, 4)
                else:
                    one(0, 8)

            # ---- schedule ----
            # Stage lags over half-groups: at hg k the loop emits QK+exp(k)
            # first (its deps never depend on this iteration's pops), then
            # drains emul/PV/evac+store backlogs down to their lag targets,
            # oldest first, so every engine's in-order queue sees deps long
            # satisfied.
            unmul = []   # (g, hh, epair, emuls)  exp'd, bias-mul pending
            unpv = []    # (g, hh, epair)         biased, PV pending
            unev = []    # (g, hh, o_ps)          PV'd, evac+store pending

            def step_evac(n):
                for _ in range(n):
                    if unev:
                        emit_evac_store(*unev.pop(0))

            def step_pv(n):
                for _ in range(n):
                    if unpv:
                        g_, hh_, ep_ = unpv.pop(0)
                        o_ = emit_pv_half(g_, hh_, ep_, 0)
                        emit_pv_half(g_, hh_, ep_, 1, o_)
                        unev.append((g_, hh_, o_))

            def step_emul(n):
                for _ in range(n):
                    if unmul:
                        g_, hh_, ep_, em_ = unmul.pop(0)
                        emit_emul(ep_, [e for e in em_ if e[1] != "pool"])
                        unpv.append((g_, hh_, ep_))

            CAP = int(os.environ.get("K_CAP", "3"))
            LMUL = int(os.environ.get("K_LMUL", "1"))
            LPV = int(os.environ.get("K_LPV", "1"))
            LEV = int(os.environ.get("K_LEV", "1"))
            for g in range(NGROUPS):
                ensure_chunk(CHUNK_OF_GROUP[min(g + 2, NGROUPS - 1)])
                for hh in range(2):
                    ep, em = emit_qk_exp(g, hh,
                                         split_exp=(g == 0 and hh == 0))
                    emit_emul(ep, [e for e in em if e[1] == "pool"])
                    step_evac(min(CAP, max(0, len(unev) - LEV)))
                    step_pv(min(CAP, max(0, len(unpv) - LPV)))
                    step_emul(min(CAP, max(0, len(unmul) - LMUL)))
                    unmul.append((g, hh, ep, em))
            # drain the tail oldest-first; the final store is split so its
            # first half overlaps the last PV
            for g_, hh_, ep_, em_ in unmul:
                emit_emul(ep_, [e for e in em_ if e[1] == "pool"])
            while unmul or unpv or unev:
                step_emul(2)
                step_pv(2)
                last = len(unpv) == 0 and len(unmul) == 0 and len(unev) == 1
                if unev:
                    g_, hh_, o_ = unev.pop(0)
                    emit_evac_store(g_, hh_, o_, split=last)
                if unev:
                    g_, hh_, o_ = unev.pop(0)
                    emit_evac_store(g_, hh_, o_)

    nc.compile()
    _BUILD_CACHE["nc"] = nc
    return nc


def _pos_mlp(inputs):
    """Host-side dynamic-pos-bias MLP + Toeplitz index (f64)."""
    f64 = lambda name: np.asarray(inputs[name], np.float64)

    def layernorm(x, g, b):
        m = x.mean(-1, keepdims=True)
        v = ((x - m) ** 2).mean(-1, keepdims=True)
        return (x - m) / np.sqrt(v + 1e-5) * g + b

    h = w = 16
    bh, bw = np.meshgrid(np.arange(1 - h, h), np.arange(1 - w, w),
                         indexing="ij")
    biases = np.stack([bh, bw], -1).reshape(-1, 2).astype(np.float64)
    pos = biases @ f64("w_proj") + f64("b_proj")
    for i in (1, 2, 3):
        pos = np.maximum(
            layernorm(pos, f64(f"ln{i}_g"), f64(f"ln{i}_b")), 0.0)
        pos = pos @ f64(f"w{i}") + f64(f"b{i}")
    ch, cw = np.meshgrid(np.arange(h), np.arange(w), indexing="ij")
    coords = np.stack([ch.reshape(-1), cw.reshape(-1)])
    rel = coords[:, :, None] - coords[:, None, :]
    idx = (rel[0] + h - 1) * (2 * w - 1) + (rel[1] + w - 1)   # [n, m]
    return pos, idx


def build_in_maps(inputs):
    q = np.asarray(inputs["q"], np.float32)
    k = np.asarray(inputs["k"], np.float32)
    v = np.asarray(inputs["v"], np.float32)
    hh = int(np.asarray(inputs["h"]))
    ww = int(np.asarray(inputs["w"]))
    assert hh == 16 and ww == 16, (hh, ww)

    pos, idx = _pos_mlp(inputs)
    # bias_z[n, m, H] in z units with the -6 octave shift baked in
    bias_z = pos[idx] * W3_SCALE + BSHIFT

    def q_layout(x):
        # [128 w, 256 n, 32 d] -> [128 p=(bi,d), (g, hh, n)] fp16, *ALPHA
        x5 = (x * np.float32(ALPHA)).reshape(16, 2, 4, 256, 32)
        return np.ascontiguousarray(
            x5.transpose(2, 4, 0, 1, 3).reshape(128, 8192).astype(np.float16))

    def k_layout(x):
        # [128 w, 256 m, 32 d] -> [128 p=(bi,d), (g, hh, mb, m)] fp16
        x6 = x.reshape(16, 2, 4, 2, 128, 32)        # g hh bi mb m d
        return np.ascontiguousarray(
            x6.transpose(2, 5, 0, 1, 3, 4).reshape(128, 8192)
            .astype(np.float16))

    def v_layout(x):
        # [128 p=m, (b 128, c 2, e 33)] fp16; e==32 -> 1.0
        v4 = x.reshape(128, 2, 128, 32)             # b c p e
        out = np.ones((128, 128, 2, 33), np.float32)
        out[:, :, :, :32] = v4.transpose(2, 0, 1, 3)
        return np.ascontiguousarray(out.reshape(128, 8448).astype(np.float16))

    ident = np.eye(128, dtype=np.float32)
    in_maps = []
    for c in range(NCORES):
        bz = bias_z[:, :, c]                        # [n, m]
        # bfold[mp, (mb, n)] = bias_z[n, mb*128+mp]
        bf = np.empty((128, 2, 256), np.float32)
        for mb in range(2):
            bf[:, mb, :] = bz[:, 128 * mb:128 * mb + 128].T
        bfold = np.ascontiguousarray(bf.reshape(128, 512))
        expbd = np.ascontiguousarray(
            np.exp2(bfold.astype(np.float64) / 1024.0).astype(np.float16))
        in_maps.append({
            "qd": q_layout(q[:, c]),
            "kd": k_layout(k[:, c]),
            "vd": v_layout(v[:, c]),
            "bfold": bfold,
            "expbd": expbd,
            "identd": ident,
        })
    return in_maps


def unshard_out(raw):
    # raw [128 p, (g 16, hh 2, j 8, e 33)] fp16 -> [B, N, D] f32 (normalize)
    r5 = raw.reshape(128, 16, 2, 8, 33).astype(np.float32)  # p g hh j e
    O = r5[..., :32]
    Z = r5[..., 32]
    out = O / Z[..., None]
    # b = (g*2+hh)*4 + j//2 ; n = (j%2)*128 + p ; d = e
    o6 = out.reshape(128, 16, 2, 4, 2, 32)           # p g hh bi nb e
    return np.ascontiguousarray(
        o6.transpose(1, 2, 3, 4, 0, 5).reshape(128, 256, 32))


def kernel(**inputs):
    from concourse.bass_utils import run_bass_kernel_spmd

    nc = _build()
    in_maps = build_in_maps(inputs)
    res = run_bass_kernel_spmd(nc, in_maps, core_ids=list(range(NCORES)))
    out = np.empty((B, H, N, D), np.float32)
    for c in range(NCORES):
        out[:, c] = unshard_out(res.results[c]["out"])
    return out


# revision 32
# speedup vs baseline: 1.0309x; 1.0309x over previous
"""Windowed attention + dynamic relative position bias on 8 NeuronCores.

Shapes: q,k,v [B=128, H=8, N=256, D=32] f32; pos-MLP width P=16; h=w=16.
Sharding: head-parallel - core c computes head c for all 128 batch windows;
the per-core head is selected purely by the bias tables passed to that core
(program is SPMD-identical).

Design (multi-engine exp, fp16 IO, host normalization):
  - All math in "z units": S = alpha*qk with alpha = 1024*log2(e)/sqrt(D);
    exp(x) == 2^(Z/1024) for Z in z units.
  - The tiny dynamic-pos-bias MLP (0.005% of the FLOPs) and its Toeplitz
    gather are computed on the host (like the other host-packed constants)
    and shipped per-core as two tables in z units:
      bfold [128,(mb,n)] f32r : 1024*log2e*bias - 6144, fold-ready
      expbd [128,(mb,n)] fp16 : 2^(bias - 6), for post-exp multiplies
  - Softmax exp is split across TWO engines per tile ([128,1024] of S):
      Act: activation Exp (scale=ln2/1024) -> fp16 E
      DVE: custom fused op EXP2_BITS_ANT computing the BITS of fp16(2^z):
           Z=S+zkf; N=(Z+B)-B (magic round to 1024s); F=Z-N;
           u16=trunc(C2*F*F + Z + C1).  One DVE instr per tile; max rel
           err ~0.2%.  Src1 must be constant (zkf); the bias is PE-folded.
  - Bias application per tile: PE fold (identity-matmul accumulate of
    bfold into S; used by all DVE tiles + F-quota Act tiles) or post-exp
    fp16 multiply by expbd (on DVE or Pool).
  - No on-device normalization: PV uses ones-augmented V; raw O and Z
    columns are evacuated PSUM->SBUF (Act Copy / DVE copy; Copy shares
    the Exp act table so no table reloads), DMA'd out as fp16, and the
    host divides O/Z.  Per-tile exp scale constants differ by path but
    every softmax row lives inside one tile, so they cancel in the
    division.
  - q/k/v host-packed fp16 (q pre-scaled by alpha): halves DMA bytes.
"""

import os
import numpy as np

B, H, N, D = 128, 8, 256, 32
NCORES = 8
NGROUPS = 16
CHUNKS = [(0, 1), (2, 5), (6, 10), (11, 15)]
CHUNK_OF_GROUP = [0, 0, 1, 1, 1, 1, 2, 2, 2, 2, 2, 3, 3, 3, 3, 3]

LOG2E = 1.4426950408889634
ALPHA = float(1024.0 * LOG2E / np.sqrt(D))         # q prescale (host)
ACT_SCALE = float(np.log(2.0) / 1024.0)            # Act exp scale in z units
W3_SCALE = float(1024.0 * LOG2E)                   # bias table scale
BSHIFT = -6144.0                                   # -6 octaves, inside bfold
BMAGIC = float(1.5 * 2 ** 33)                      # fp32 round-to-1024 magic
EXP_C1 = 433.57                                    # mantissa-parabola const
EXP_C2 = 3.3007e-4                                 # mantissa-parabola curv
ZKF_CONST = 12800.0 + 6144.0                       # folded-DVE zbias const

# tile modes: 'A' = Act exp + emul; 'F' = Act exp + PE fold;
#             'D' = DVE custom exp + PE fold.
X_CNT = int(os.environ.get("K_X", "21"))           # DVE-exp tiles (of 64)
F_CNT = int(os.environ.get("K_F", "11"))            # PE-fold Act tiles
MDVE_CNT = int(os.environ.get("K_MDVE", "22"))     # DVE-emul quota (rest Pool)
EVA_ACT = int(os.environ.get("K_EVA", "10"))        # PSUM evacs on Act (of 32)

_BUILD_CACHE = {}


def _tile_modes():
    """Assign A/F/D to each of the 64 tiles, plus emul engine for A."""
    modes = [None] * 64
    acc_d = acc_f = 0
    for i in range(64):
        nd = ((i + 1) * X_CNT) // 64
        if nd > acc_d:
            modes[i] = "D"
            acc_d = nd
        else:
            nf = ((i + 1) * F_CNT) // 64
            if nf > acc_f:
                modes[i] = "F"
                acc_f = nf
            else:
                modes[i] = "A"
    for t in (62, 63):
        if modes[t] == "A":
            modes[t] = "F"
    # very first tile on Act (fastest start)
    if modes[0] != "A":
        swap = next(t for t in range(1, 64) if modes[t] == "A")
        modes[swap] = modes[0]
        modes[0] = "A"
    eng = {}
    need = [t for t in range(64) if modes[t] == "A"]
    nm = max(len(need), 1)
    for j, t in enumerate(need):
        if ((j + 1) * MDVE_CNT) // nm > (j * MDVE_CNT) // nm:
            eng[t] = "dve"
        else:
            eng[t] = "pool"
    return modes, eng


def _register_exp_op():
    if "op" in _BUILD_CACHE:
        return _BUILD_CACHE["op"]
    from concourse.dve_spec import Spec, Src0, Src1, C0, C1, C2, lower
    from concourse import dve_ops
    from concourse.dve_table_gen import dve_ver_for
    from concourse.dve_uop import DveOpSpec

    for o in dve_ops.OPS:
        if o.name == "EXP2_BITS_ANT":
            _BUILD_CACHE["op"] = o
            return o

    Z = Src0 + Src1
    Nq = (Z + C0) - C0
    F = Z - Nq
    body = (C2 * F) * F + (Z + C1)

    def ref(in0, in1, s0, s1, imm2):
        f32 = np.float32
        Zv = f32(f32(in0.astype(np.float32)) + f32(in1.astype(np.float32)))
        t = f32(Zv + f32(s0))
        Nv = f32(t - f32(s0))
        Fv = f32(Zv - Nv)
        u = f32(f32(f32(f32(imm2)) * Fv) * Fv + f32(Zv + f32(s1)))
        return np.clip(u, 0.0, 65535.0)

    spec = Spec(body=body, reference=ref)
    ver = dve_ver_for("TRN2")
    row = dve_ops._CUSTOM_DVE_ROW_BASE + len(dve_ops.OPS)
    sha = DveOpSpec(name="EXP2_BITS_ANT", opcode=row,
                    uops=lower(spec, ver=ver), rd1_en=True).sha(ver)
    op = dve_ops.DveOp("EXP2_BITS_ANT", spec, subdim=False,
                       uops_sha={ver: sha})
    dve_ops.OPS.append(op)
    dve_ops.CUSTOM_DVE_SPECS[op.name] = spec
    dve_ops._SUB_OPCODE_FOR_NAME[op.name] = row
    _BUILD_CACHE["op"] = op
    return op


def _build():
    if "nc" in _BUILD_CACHE:
        return _BUILD_CACHE["nc"]
    import concourse.bacc as bacc
    import concourse.mybir as mybir
    from concourse.tile import TileContext
    from bass_rust import AP

    exp_op = _register_exp_op()

    F32 = mybir.dt.float32
    F32R = mybir.dt.float32r
    FP16 = mybir.dt.float16
    U16 = mybir.dt.uint16
    AF = mybir.ActivationFunctionType

    nc = bacc.Bacc("TRN2", target_bir_lowering=False, debug=False,
                   num_devices=NCORES)

    # host-prearranged layouts (see build_in_maps), all fp16:
    # qd [128 p=(bi,d), (g 16, hh 2, n 256)] fp16, pre-scaled by ALPHA
    # kd [128 p=(bi,d), (g 16, hh 2, mb 2, m 128)] fp16
    # vd [128 p=m, (b 128, c 2, e 33)] fp16 (e==32 -> 1.0)
    qd = nc.dram_tensor("qd", [128, 8192], FP16, kind="ExternalInput")
    kd = nc.dram_tensor("kd", [128, 8192], FP16, kind="ExternalInput")
    vd = nc.dram_tensor("vd", [128, 8448], FP16, kind="ExternalInput")
    bfold_d = nc.dram_tensor("bfold", [128, 512], F32R, kind="ExternalInput")
    expb_d = nc.dram_tensor("expbd", [128, 512], FP16, kind="ExternalInput")
    ident_d = nc.dram_tensor("identd", [128, 128], F32, kind="ExternalInput")

    # raw O (32 cols) + Z (1 col) per j, 8 j per half-group, fp16
    out_d = nc.dram_tensor("out", [128, 8448], FP16, kind="ExternalOutput")

    MODES, EMUL_ENG = _tile_modes()

    with TileContext(nc) as tc:
        with (
            tc.tile_pool(name="const", bufs=1) as constp,
            tc.tile_pool(name="vpool", bufs=1) as vpool,
            tc.tile_pool(name="epool", bufs=int(os.environ.get("K_EP", "28"))) as epool,
            tc.tile_pool(name="spsum", bufs=int(os.environ.get("K_SB", "3")), space="PSUM") as spsum,
            tc.tile_pool(name="auxpsum", bufs=int(os.environ.get("K_AB", "2")), space="PSUM") as auxpsum,
        ):
            # ---- full-size q/k/v SBUF tiles; chunked loads emitted lazily
            q_all = vpool.tile([128, 8192], FP16)
            k_all = vpool.tile([128, 8192], FP16)
            v_all = vpool.tile([128, 8448], FP16)

            chunk_loaded = [False] * len(CHUNKS)

            def emit_chunk(ci):
                g0, g1 = CHUNKS[ci]
                ng = g1 - g0 + 1
                qk0 = g0
                if ci == 0:
                    qk0 = 1      # group 0 of q/k loaded via the fast path
                # q on Pool SWDGE; k/v on sync (HWDGE)
                nc.gpsimd.dma_start(
                    q_all[:, 512 * qk0:512 * (g1 + 1)],
                    AP(qd, 512 * qk0,
                       [[8192, 128], [1, 512 * (g1 - qk0 + 1)]]))
                nc.sync.dma_start(
                    k_all[:, 512 * qk0:512 * (g1 + 1)],
                    AP(kd, 512 * qk0,
                       [[8192, 128], [1, 512 * (g1 - qk0 + 1)]]))
                nc.sync.dma_start(
                    v_all[:, 528 * g0:528 * (g1 + 1)],
                    AP(vd, 528 * g0, [[8448, 128], [1, 528 * ng]]))

            def ensure_chunk(ci):
                if not chunk_loaded[ci]:
                    chunk_loaded[ci] = True
                    emit_chunk(ci)

            # fast path for the very first QK: k via sync HWDGE, q via Pool
            # SWDGE - different dispatchers run concurrently
            nc.sync.dma_start(k_all[:, 0:256],
                              AP(kd, 0, [[8192, 128], [1, 256]]))
            nc.gpsimd.dma_start(q_all[:, 0:256],
                                AP(qd, 0, [[8192, 128], [1, 256]]))
            nc.sync.dma_start(k_all[:, 256:512],
                              AP(kd, 256, [[8192, 128], [1, 256]]))
            nc.gpsimd.dma_start(q_all[:, 256:512],
                                AP(qd, 256, [[8192, 128], [1, 256]]))

            # bias tables + fold identity (scalar queue: its seq is free
            # early and the transfers are small)
            zkf = constp.tile([128, 1024], F32)
            nc.gpsimd.memset(zkf[:, :], ZKF_CONST)
            bfold = constp.tile([128, 512], F32R)
            nc.scalar.dma_start(bfold[:, :], bfold_d[:, :])
            expb = constp.tile([128, 512], FP16)
            nc.scalar.dma_start(expb[:, :], expb_d[:, :])
            ident_f = constp.tile([128, 128], F32)
            nc.scalar.dma_start(ident_f[:, :], ident_d[:, :])
            ident_r = constp.tile([128, 128], F32R)
            nc.vector.tensor_copy(ident_r[:, :], ident_f[:, :])

            ensure_chunk(0)
            ensure_chunk(1)

            # --- main pipeline over 64 tiles (2 per half-group) ---
            def emit_qk_exp(g, hh, split_exp=False):
                """QK matmuls (+bias folds) + exp -> epair, pending emuls."""
                ho = 512 * g + 256 * hh
                epair = []
                emuls = []
                for half in range(2):
                    t_idx = 4 * g + 2 * hh + half
                    mode = MODES[t_idx]
                    fold = mode != "A"
                    sp = spsum.tile([128, 1024], F32, tag="S",
                                    name=f"s{g}_{hh}_{half}")
                    for bi2 in range(2):
                        bi = 2 * half + bi2
                        fo = 512 * bi2
                        for mb in range(2):
                            out_ap = sp[:, fo + 256 * mb:fo + 256 * mb + 256]
                            nc.tensor.matmul(
                                out_ap,
                                k_all[32 * bi:32 * bi + 32,
                                      ho + 128 * mb:ho + 128 * mb + 128],
                                q_all[32 * bi:32 * bi + 32, ho:ho + 256],
                                tile_position=(32 * bi, 0),
                                start=True, stop=not fold)
                            if fold:
                                nc.tensor.matmul(
                                    out_ap, ident_r[:, :],
                                    bfold[:, 256 * mb:256 * mb + 256],
                                    tile_position=(0, 0),
                                    start=False, stop=True)
                    e = epool.tile([128, 1024], FP16, tag="E",
                                   name=f"e{g}_{hh}_{half}")
                    if mode == "D":
                        nc.vector._custom_dve(
                            exp_op, out=e[:, :].bitcast(U16),
                            in0=sp[:, :], in1=zkf[:, :],
                            s0=BMAGIC, s1=EXP_C1, imm2=EXP_C2)
                    else:
                        if split_exp:
                            nc.scalar.activation(e[:, :512], sp[:, :512],
                                                 AF.Exp, scale=ACT_SCALE)
                            nc.scalar.activation(e[:, 512:], sp[:, 512:],
                                                 AF.Exp, scale=ACT_SCALE)
                        else:
                            nc.scalar.activation(e[:, :], sp[:, :], AF.Exp,
                                                 scale=ACT_SCALE)
                        if mode == "A":
                            emuls.append((half, EMUL_ENG[t_idx]))
                    epair.append(e)
                return epair, emuls

            def emit_emul(epair, emuls):
                for half, eng in emuls:
                    e = epair[half]
                    e4 = e[:, :].rearrange("p (j mb n) -> p j mb n",
                                           mb=2, n=256)
                    bb = (expb[:, :].rearrange("p (mb n) -> p mb n", n=256)
                          .unsqueeze(1).broadcast_to((128, 2, 2, 256)))
                    if eng == "pool":
                        nc.gpsimd.tensor_mul(e4, e4, bb)
                    else:
                        nc.vector.tensor_mul(e4, e4, bb)

            def emit_pv_half(g, hh, epair, half, o_ps=None):
                if o_ps is None:
                    o_ps = auxpsum.tile([128, 264], F32, tag="aux2",
                                        name=f"ops{g}_{hh}")
                e = epair[half]
                for bi2 in range(2):
                    bi = 2 * half + bi2
                    fo = 512 * bi2
                    vb = 66 * (8 * g + 4 * hh + bi)
                    for nb in range(2):
                        j = 2 * bi + nb
                        for c in range(2):
                            nc.tensor.matmul(
                                o_ps[:, 33 * j:33 * j + 33],
                                e[:, fo + 256 * c + 128 * nb:
                                  fo + 256 * c + 128 * nb + 128],
                                v_all[:, vb + 33 * c:vb + 33 * c + 33],
                                start=(c == 0), stop=(c == 1))
                return o_ps

            evac_ctr = [0]

            def emit_evac_store(g, hh, o_ps, split=False):
                osb = epool.tile([128, 264], FP16, tag="osb",
                                 name=f"osb{g}_{hh}")

                def one(j0, nj):
                    src_ = o_ps[:, 33 * j0:33 * (j0 + nj)]
                    dst_ = osb[:, 33 * j0:33 * (j0 + nj)]
                    i = evac_ctr[0]
                    evac_ctr[0] += 1
                    if ((i + 1) * EVA_ACT) // 32 > (i * EVA_ACT) // 32:
                        nc.scalar.activation(dst_, src_, AF.Copy)
                    else:
                        nc.vector.tensor_copy(dst_, src_)
                    nc.sync.dma_start(
                        AP(out_d, 264 * (2 * g + hh) + 33 * j0,
                           [[8448, 128], [1, 33 * nj]]),
                        osb[:, 33 * j0:33 * (j0 + nj)])

                if split:
                    one(0, 4)
                    one(4, 4)
                else:
                    one(0, 8)

            # ---- schedule ----
            # Stage lags over half-groups: at hg k the loop emits QK+exp(k)
            # first (its deps never depend on this iteration's pops), then
            # drains emul/PV/evac+store backlogs down to their lag targets,
            # oldest first, so every engine's in-order queue sees deps long
            # satisfied.
            unmul = []   # (g, hh, epair, emuls)  exp'd, bias-mul pending
            unpv = []    # (g, hh, epair)         biased, PV pending
            unev = []    # (g, hh, o_ps)          PV'd, evac+store pending

            def step_evac(n):
                for _ in range(n):
                    if unev:
                        emit_evac_store(*unev.pop(0))

            def step_pv(n):
                for _ in range(n):
                    if unpv:
                        g_, hh_, ep_ = unpv.pop(0)
                        o_ = emit_pv_half(g_, hh_, ep_, 0)
                        emit_pv_half(g_, hh_, ep_, 1, o_)
                        unev.append((g_, hh_, o_))

            def step_emul(n):
                for _ in range(n):
                    if unmul:
                        g_, hh_, ep_, em_ = unmul.pop(0)
                        emit_emul(ep_, em_)
                        unpv.append((g_, hh_, ep_))

            CAP = int(os.environ.get("K_CAP", "3"))
            LMUL = int(os.environ.get("K_LMUL", "1"))
            LPV = int(os.environ.get("K_LPV", "1"))
            LEV = int(os.environ.get("K_LEV", "1"))
            for g in range(NGROUPS):
                ensure_chunk(CHUNK_OF_GROUP[min(g + 2, NGROUPS - 1)])
                for hh in range(2):
                    ep, em = emit_qk_exp(g, hh,
                                         split_exp=(g == 0 and hh == 0))
                    step_evac(min(CAP, max(0, len(unev) - LEV)))
                    step_pv(min(CAP, max(0, len(unpv) - LPV)))
                    step_emul(min(CAP, max(0, len(unmul) - LMUL)))
                    unmul.append((g, hh, ep, em))
            # drain the tail oldest-first; the final store is split so its
            # first half overlaps the last PV
            while unmul or unpv or unev:
                step_emul(2)
                step_pv(2)
                last = len(unpv) == 0 and len(unmul) == 0 and len(unev) == 1
                if unev:
                    g_, hh_, o_ = unev.pop(0)
                    emit_evac_store(g_, hh_, o_, split=last)
                if unev:
                    g_, hh_, o_ = unev.pop(0)
                    emit_evac_store(g_, hh_, o_)

    nc.compile()
    _BUILD_CACHE["nc"] = nc
    return nc


def _pos_mlp(inputs):
    """Host-side dynamic-pos-bias MLP + Toeplitz index (f64)."""
    f64 = lambda name: np.asarray(inputs[name], np.float64)

    def layernorm(x, g, b):
        m = x.mean(-1, keepdims=True)
        v = ((x - m) ** 2).mean(-1, keepdims=True)
        return (x - m) / np.sqrt(v + 1e-5) * g + b

    h = w = 16
    bh, bw = np.meshgrid(np.arange(1 - h, h), np.arange(1 - w, w),
                         indexing="ij")
    biases = np.stack([bh, bw], -1).reshape(-1, 2).astype(np.float64)
    pos = biases @ f64("w_proj") + f64("b_proj")
    for i in (1, 2, 3):
        pos = np.maximum(
            layernorm(pos, f64(f"ln{i}_g"), f64(f"ln{i}_b")), 0.0)
        pos = pos @ f64(f"w{i}") + f64(f"b{i}")
    ch, cw = np.meshgrid(np.arange(h), np.arange(w), indexing="ij")
    coords = np.stack([ch.reshape(-1), cw.reshape(-1)])
    rel = coords[:, :, None] - coords[:, None, :]
    idx = (rel[0] + h - 1) * (2 * w - 1) + (rel[1] + w - 1)   # [n, m]
    return pos, idx


def build_in_maps(inputs):
    q = np.asarray(inputs["q"], np.float32)
    k = np.asarray(inputs["k"], np.float32)
    v = np.asarray(inputs["v"], np.float32)
    hh = int(np.asarray(inputs["h"]))
    ww = int(np.asarray(inputs["w"]))
    assert hh == 16 and ww == 16, (hh, ww)

    pos, idx = _pos_mlp(inputs)
    # bias_z[n, m, H] in z units with the -6 octave shift baked in
    bias_z = pos[idx] * W3_SCALE + BSHIFT

    def q_layout(x):
        # [128 w, 256 n, 32 d] -> [128 p=(bi,d), (g, hh, n)] fp16, *ALPHA
        x5 = (x * np.float32(ALPHA)).reshape(16, 2, 4, 256, 32)
        return np.ascontiguousarray(
            x5.transpose(2, 4, 0, 1, 3).reshape(128, 8192).astype(np.float16))

    def k_layout(x):
        # [128 w, 256 m, 32 d] -> [128 p=(bi,d), (g, hh, mb, m)] fp16
        x6 = x.reshape(16, 2, 4, 2, 128, 32)        # g hh bi mb m d
        return np.ascontiguousarray(
            x6.transpose(2, 5, 0, 1, 3, 4).reshape(128, 8192)
            .astype(np.float16))

    def v_layout(x):
        # [128 p=m, (b 128, c 2, e 33)] fp16; e==32 -> 1.0
        v4 = x.reshape(128, 2, 128, 32)             # b c p e
        out = np.ones((128, 128, 2, 33), np.float32)
        out[:, :, :, :32] = v4.transpose(2, 0, 1, 3)
        return np.ascontiguousarray(out.reshape(128, 8448).astype(np.float16))

    ident = np.eye(128, dtype=np.float32)
    in_maps = []
    for c in range(NCORES):
        bz = bias_z[:, :, c]                        # [n, m]
        # bfold[mp, (mb, n)] = bias_z[n, mb*128+mp]
        bf = np.empty((128, 2, 256), np.float32)
        for mb in range(2):
            bf[:, mb, :] = bz[:, 128 * mb:128 * mb + 128].T
        bfold = np.ascontiguousarray(bf.reshape(128, 512))
        expbd = np.ascontiguousarray(
            np.exp2(bfold.astype(np.float64) / 1024.0).astype(np.float16))
        in_maps.append({
            "qd": q_layout(q[:, c]),
            "kd": k_layout(k[:, c]),
            "vd": v_layout(v[:, c]),
            "bfold": bfold,
            "expbd": expbd,
            "identd": ident,
        })
    return in_maps


def unshard_out(raw):
    # raw [128 p, (g 16, hh 2, j 8, e 33)] fp16 -> [B, N, D] f32 (normalize)
    r5 = raw.reshape(128, 16, 2, 8, 33).astype(np.float32)  # p g hh j e
    O = r5[..., :32]
    Z = r5[..., 32]
    out = O / Z[..., None]
    # b = (g*2+hh)*4 + j//2 ; n = (j%2)*128 + p ; d = e
    o6 = out.reshape(128, 16, 2, 4, 2, 32)           # p g hh bi nb e
    return np.ascontiguousarray(
        o6.transpose(1, 2, 3, 4, 0, 5).reshape(128, 256, 32))


def kernel(**inputs):
    from concourse.bass_utils import run_bass_kernel_spmd

    nc = _build()
    in_maps = build_in_maps(inputs)
    res = run_bass_kernel_spmd(nc, in_maps, core_ids=list(range(NCORES)))
    out = np.empty((B, H, N, D), np.float32)
    for c in range(NCORES):
        out[:, c] = unshard_out(res.results[c]["out"])
    return out


# revision 33
# speedup vs baseline: 1.0515x; 1.0199x over previous
"""Windowed attention + dynamic relative position bias on 8 NeuronCores.

Shapes: q,k,v [B=128, H=8, N=256, D=32] f32; pos-MLP width P=16; h=w=16.
Sharding: head-parallel - core c computes head c for all 128 batch windows;
the per-core head is selected purely by the bias tables passed to that core
(program is SPMD-identical).

Design (multi-engine exp, fp16 IO, host normalization):
  - All math in "z units": S = alpha*qk with alpha = 1024*log2(e)/sqrt(D);
    exp(x) == 2^(Z/1024) for Z in z units.
  - The tiny dynamic-pos-bias MLP (0.005% of the FLOPs) and its Toeplitz
    gather are computed on the host (like the other host-packed constants)
    and shipped per-core as two tables in z units:
      bfold [128,(mb,n)] f32r : 1024*log2e*bias - 6144, fold-ready
      expbd [128,(mb,n)] fp16 : 2^(bias - 6), for post-exp multiplies
  - Softmax exp is split across TWO engines per tile ([128,1024] of S):
      Act: activation Exp (scale=ln2/1024) -> fp16 E
      DVE: custom fused op EXP2_BITS_ANT computing the BITS of fp16(2^z):
           Z=S+zkf; N=(Z+B)-B (magic round to 1024s); F=Z-N;
           u16=trunc(C2*F*F + Z + C1).  One DVE instr per tile; max rel
           err ~0.2%.  Src1 must be constant (zkf); the bias is PE-folded.
  - Bias application per tile: PE fold (identity-matmul accumulate of
    bfold into S; used by all DVE tiles + F-quota Act tiles) or post-exp
    fp16 multiply by expbd (on DVE or Pool).
  - No on-device normalization: PV uses ones-augmented V; raw O and Z
    columns are evacuated PSUM->SBUF (Act Copy / DVE copy; Copy shares
    the Exp act table so no table reloads), DMA'd out as fp16, and the
    host divides O/Z.  Per-tile exp scale constants differ by path but
    every softmax row lives inside one tile, so they cancel in the
    division.
  - q/k/v host-packed fp16 (q pre-scaled by alpha): halves DMA bytes.
"""

import os
import numpy as np

B, H, N, D = 128, 8, 256, 32
NCORES = 8
NGROUPS = 16
CHUNKS = [(0, 1), (2, 5), (6, 10), (11, 15)]
CHUNK_OF_GROUP = [0, 0, 1, 1, 1, 1, 2, 2, 2, 2, 2, 3, 3, 3, 3, 3]

LOG2E = 1.4426950408889634
ALPHA = float(1024.0 * LOG2E / np.sqrt(D))         # q prescale (host)
ACT_SCALE = float(np.log(2.0) / 1024.0)            # Act exp scale in z units
W3_SCALE = float(1024.0 * LOG2E)                   # bias table scale
BSHIFT = -6144.0                                   # -6 octaves, inside bfold
BMAGIC = float(1.5 * 2 ** 33)                      # fp32 round-to-1024 magic
EXP_C1 = 433.57                                    # mantissa-parabola const
EXP_C2 = 3.3007e-4                                 # mantissa-parabola curv
ZKF_CONST = 12800.0 + 6144.0                       # folded-DVE zbias const

# tile modes: 'A' = Act exp + emul; 'F' = Act exp + PE fold;
#             'D' = DVE custom exp + PE fold.
X_CNT = int(os.environ.get("K_X", "21"))           # DVE-exp tiles (of 64)
F_CNT = int(os.environ.get("K_F", "11"))            # PE-fold Act tiles
MDVE_CNT = int(os.environ.get("K_MDVE", "22"))     # DVE-emul quota (rest Pool)
EVA_ACT = int(os.environ.get("K_EVA", "10"))        # PSUM evacs on Act (of 32)

_BUILD_CACHE = {}


def _tile_modes():
    """Assign A/F/D to each of the 64 tiles, plus emul engine for A."""
    modes = [None] * 64
    acc_d = acc_f = 0
    for i in range(64):
        nd = ((i + 1) * X_CNT) // 64
        if nd > acc_d:
            modes[i] = "D"
            acc_d = nd
        else:
            nf = ((i + 1) * F_CNT) // 64
            if nf > acc_f:
                modes[i] = "F"
                acc_f = nf
            else:
                modes[i] = "A"
    # very first tile on Act (fastest start)
    if modes[0] != "A":
        swap = next(t for t in range(1, 64) if modes[t] == "A")
        modes[swap] = modes[0]
        modes[0] = "A"
    eng = {}
    need = [t for t in range(64) if modes[t] == "A"]
    nm = max(len(need), 1)
    for j, t in enumerate(need):
        if ((j + 1) * MDVE_CNT) // nm > (j * MDVE_CNT) // nm:
            eng[t] = "dve"
        else:
            eng[t] = "pool"
    return modes, eng


def _register_exp_op():
    if "op" in _BUILD_CACHE:
        return _BUILD_CACHE["op"]
    from concourse.dve_spec import Spec, Src0, Src1, C0, C1, C2, lower
    from concourse import dve_ops
    from concourse.dve_table_gen import dve_ver_for
    from concourse.dve_uop import DveOpSpec

    for o in dve_ops.OPS:
        if o.name == "EXP2_BITS_ANT":
            _BUILD_CACHE["op"] = o
            return o

    Z = Src0 + Src1
    Nq = (Z + C0) - C0
    F = Z - Nq
    body = (C2 * F) * F + (Z + C1)

    def ref(in0, in1, s0, s1, imm2):
        f32 = np.float32
        Zv = f32(f32(in0.astype(np.float32)) + f32(in1.astype(np.float32)))
        t = f32(Zv + f32(s0))
        Nv = f32(t - f32(s0))
        Fv = f32(Zv - Nv)
        u = f32(f32(f32(f32(imm2)) * Fv) * Fv + f32(Zv + f32(s1)))
        return np.clip(u, 0.0, 65535.0)

    spec = Spec(body=body, reference=ref)
    ver = dve_ver_for("TRN2")
    row = dve_ops._CUSTOM_DVE_ROW_BASE + len(dve_ops.OPS)
    sha = DveOpSpec(name="EXP2_BITS_ANT", opcode=row,
                    uops=lower(spec, ver=ver), rd1_en=True).sha(ver)
    op = dve_ops.DveOp("EXP2_BITS_ANT", spec, subdim=False,
                       uops_sha={ver: sha})
    dve_ops.OPS.append(op)
    dve_ops.CUSTOM_DVE_SPECS[op.name] = spec
    dve_ops._SUB_OPCODE_FOR_NAME[op.name] = row
    _BUILD_CACHE["op"] = op
    return op


def _build():
    if "nc" in _BUILD_CACHE:
        return _BUILD_CACHE["nc"]
    import concourse.bacc as bacc
    import concourse.mybir as mybir
    from concourse.tile import TileContext
    from bass_rust import AP

    exp_op = _register_exp_op()

    F32 = mybir.dt.float32
    F32R = mybir.dt.float32r
    FP16 = mybir.dt.float16
    U16 = mybir.dt.uint16
    AF = mybir.ActivationFunctionType

    nc = bacc.Bacc("TRN2", target_bir_lowering=False, debug=False,
                   num_devices=NCORES)

    # host-prearranged layouts (see build_in_maps), all fp16:
    # qd [128 p=(bi,d), (g 16, hh 2, n 256)] fp16, pre-scaled by ALPHA
    # kd [128 p=(bi,d), (g 16, hh 2, mb 2, m 128)] fp16
    # vd [128 p=m, (b 128, c 2, e 33)] fp16 (e==32 -> 1.0)
    qd = nc.dram_tensor("qd", [128, 8192], FP16, kind="ExternalInput")
    kd = nc.dram_tensor("kd", [128, 8192], FP16, kind="ExternalInput")
    vd = nc.dram_tensor("vd", [128, 8448], FP16, kind="ExternalInput")
    bfold_d = nc.dram_tensor("bfold", [128, 512], F32R, kind="ExternalInput")
    expb_d = nc.dram_tensor("expbd", [128, 512], FP16, kind="ExternalInput")
    ident_d = nc.dram_tensor("identd", [128, 128], F32, kind="ExternalInput")

    # raw O (32 cols) + Z (1 col) per j, 8 j per half-group, fp16
    out_d = nc.dram_tensor("out", [128, 8448], FP16, kind="ExternalOutput")

    MODES, EMUL_ENG = _tile_modes()

    with TileContext(nc) as tc:
        with (
            tc.tile_pool(name="const", bufs=1) as constp,
            tc.tile_pool(name="vpool", bufs=1) as vpool,
            tc.tile_pool(name="epool", bufs=int(os.environ.get("K_EP", "28"))) as epool,
            tc.tile_pool(name="spsum", bufs=int(os.environ.get("K_SB", "3")), space="PSUM") as spsum,
            tc.tile_pool(name="auxpsum", bufs=int(os.environ.get("K_AB", "2")), space="PSUM") as auxpsum,
        ):
            # ---- full-size q/k/v SBUF tiles; chunked loads emitted lazily
            q_all = vpool.tile([128, 8192], FP16)
            k_all = vpool.tile([128, 8192], FP16)
            v_all = vpool.tile([128, 8448], FP16)

            chunk_loaded = [False] * len(CHUNKS)

            def emit_chunk(ci):
                g0, g1 = CHUNKS[ci]
                ng = g1 - g0 + 1
                qk0 = g0
                if ci == 0:
                    qk0 = 1      # group 0 of q/k loaded via the fast path
                # q on Pool SWDGE; k/v on sync (HWDGE)
                nc.gpsimd.dma_start(
                    q_all[:, 512 * qk0:512 * (g1 + 1)],
                    AP(qd, 512 * qk0,
                       [[8192, 128], [1, 512 * (g1 - qk0 + 1)]]))
                nc.sync.dma_start(
                    k_all[:, 512 * qk0:512 * (g1 + 1)],
                    AP(kd, 512 * qk0,
                       [[8192, 128], [1, 512 * (g1 - qk0 + 1)]]))
                nc.sync.dma_start(
                    v_all[:, 528 * g0:528 * (g1 + 1)],
                    AP(vd, 528 * g0, [[8448, 128], [1, 528 * ng]]))

            def ensure_chunk(ci):
                if not chunk_loaded[ci]:
                    chunk_loaded[ci] = True
                    emit_chunk(ci)

            # fast path for the very first QK: k via sync HWDGE, q via Pool
            # SWDGE - different dispatchers run concurrently
            nc.sync.dma_start(k_all[:, 0:256],
                              AP(kd, 0, [[8192, 128], [1, 256]]))
            nc.gpsimd.dma_start(q_all[:, 0:256],
                                AP(qd, 0, [[8192, 128], [1, 256]]))
            nc.sync.dma_start(k_all[:, 256:512],
                              AP(kd, 256, [[8192, 128], [1, 256]]))
            nc.gpsimd.dma_start(q_all[:, 256:512],
                                AP(qd, 256, [[8192, 128], [1, 256]]))

            # bias tables + fold identity (scalar queue: its seq is free
            # early and the transfers are small)
            zkf = constp.tile([128, 1024], F32)
            nc.gpsimd.memset(zkf[:, :], ZKF_CONST)
            bfold = constp.tile([128, 512], F32R)
            nc.scalar.dma_start(bfold[:, :], bfold_d[:, :])
            expb = constp.tile([128, 512], FP16)
            nc.scalar.dma_start(expb[:, :], expb_d[:, :])
            ident_f = constp.tile([128, 128], F32)
            nc.scalar.dma_start(ident_f[:, :], ident_d[:, :])
            ident_r = constp.tile([128, 128], F32R)
            nc.vector.tensor_copy(ident_r[:, :], ident_f[:, :])

            ensure_chunk(0)
            ensure_chunk(1)

            # --- main pipeline over 64 tiles (2 per half-group) ---
            def emit_qk_exp(g, hh, split_exp=False):
                """QK matmuls (+bias folds) + exp -> epair, pending emuls."""
                ho = 512 * g + 256 * hh
                epair = []
                emuls = []
                for half in range(2):
                    t_idx = 4 * g + 2 * hh + half
                    mode = MODES[t_idx]
                    fold = mode != "A"
                    sp = spsum.tile([128, 1024], F32, tag="S",
                                    name=f"s{g}_{hh}_{half}")
                    for bi2 in range(2):
                        bi = 2 * half + bi2
                        fo = 512 * bi2
                        for mb in range(2):
                            out_ap = sp[:, fo + 256 * mb:fo + 256 * mb + 256]
                            nc.tensor.matmul(
                                out_ap,
                                k_all[32 * bi:32 * bi + 32,
                                      ho + 128 * mb:ho + 128 * mb + 128],
                                q_all[32 * bi:32 * bi + 32, ho:ho + 256],
                                tile_position=(32 * bi, 0),
                                start=True, stop=not fold)
                            if fold:
                                nc.tensor.matmul(
                                    out_ap, ident_r[:, :],
                                    bfold[:, 256 * mb:256 * mb + 256],
                                    tile_position=(0, 0),
                                    start=False, stop=True)
                    e = epool.tile([128, 1024], FP16, tag="E",
                                   name=f"e{g}_{hh}_{half}")
                    if mode == "D":
                        nc.vector._custom_dve(
                            exp_op, out=e[:, :].bitcast(U16),
                            in0=sp[:, :], in1=zkf[:, :],
                            s0=BMAGIC, s1=EXP_C1, imm2=EXP_C2)
                    else:
                        if split_exp:
                            nc.scalar.activation(e[:, :512], sp[:, :512],
                                                 AF.Exp, scale=ACT_SCALE)
                            nc.scalar.activation(e[:, 512:], sp[:, 512:],
                                                 AF.Exp, scale=ACT_SCALE)
                        else:
                            nc.scalar.activation(e[:, :], sp[:, :], AF.Exp,
                                                 scale=ACT_SCALE)
                        if mode == "A":
                            emuls.append((half, EMUL_ENG[t_idx]))
                    epair.append(e)
                return epair, emuls

            def emit_emul(epair, emuls):
                for half, eng in emuls:
                    e = epair[half]
                    e4 = e[:, :].rearrange("p (j mb n) -> p j mb n",
                                           mb=2, n=256)
                    bb = (expb[:, :].rearrange("p (mb n) -> p mb n", n=256)
                          .unsqueeze(1).broadcast_to((128, 2, 2, 256)))
                    if eng == "pool":
                        nc.gpsimd.tensor_mul(e4, e4, bb)
                    else:
                        nc.vector.tensor_mul(e4, e4, bb)

            def emit_pv_half(g, hh, epair, half, o_ps=None):
                if o_ps is None:
                    o_ps = auxpsum.tile([128, 264], F32, tag="aux2",
                                        name=f"ops{g}_{hh}")
                e = epair[half]
                for bi2 in range(2):
                    bi = 2 * half + bi2
                    fo = 512 * bi2
                    vb = 66 * (8 * g + 4 * hh + bi)
                    for nb in range(2):
                        j = 2 * bi + nb
                        for c in range(2):
                            nc.tensor.matmul(
                                o_ps[:, 33 * j:33 * j + 33],
                                e[:, fo + 256 * c + 128 * nb:
                                  fo + 256 * c + 128 * nb + 128],
                                v_all[:, vb + 33 * c:vb + 33 * c + 33],
                                start=(c == 0), stop=(c == 1))
                return o_ps

            evac_ctr = [0]

            def emit_evac_store(g, hh, o_ps, split=False):
                osb = epool.tile([128, 264], FP16, tag="osb",
                                 name=f"osb{g}_{hh}")

                def one(j0, nj):
                    src_ = o_ps[:, 33 * j0:33 * (j0 + nj)]
                    dst_ = osb[:, 33 * j0:33 * (j0 + nj)]
                    i = evac_ctr[0]
                    evac_ctr[0] += 1
                    if ((i + 1) * EVA_ACT) // 32 > (i * EVA_ACT) // 32:
                        nc.scalar.activation(dst_, src_, AF.Copy)
                    else:
                        nc.vector.tensor_copy(dst_, src_)
                    nc.sync.dma_start(
                        AP(out_d, 264 * (2 * g + hh) + 33 * j0,
                           [[8448, 128], [1, 33 * nj]]),
                        osb[:, 33 * j0:33 * (j0 + nj)])

                if split:
                    one(0, 4)
                    one(4, 4)
                else:
                    one(0, 8)

            # ---- schedule ----
            # Stage lags over half-groups: at hg k the loop emits QK+exp(k)
            # first (its deps never depend on this iteration's pops), then
            # drains emul/PV/evac+store backlogs down to their lag targets,
            # oldest first, so every engine's in-order queue sees deps long
            # satisfied.
            unmul = []   # (g, hh, epair, emuls)  exp'd, bias-mul pending
            unpv = []    # (g, hh, epair)         biased, PV pending
            unev = []    # (g, hh, o_ps)          PV'd, evac+store pending

            def step_evac(n):
                for _ in range(n):
                    if unev:
                        emit_evac_store(*unev.pop(0))

            def step_pv(n):
                for _ in range(n):
                    if unpv:
                        g_, hh_, ep_ = unpv.pop(0)
                        o_ = emit_pv_half(g_, hh_, ep_, 0)
                        emit_pv_half(g_, hh_, ep_, 1, o_)
                        unev.append((g_, hh_, o_))

            def step_emul(n):
                for _ in range(n):
                    if unmul:
                        g_, hh_, ep_, em_ = unmul.pop(0)
                        emit_emul(ep_, em_)
                        unpv.append((g_, hh_, ep_))

            CAP = int(os.environ.get("K_CAP", "3"))
            LMUL = int(os.environ.get("K_LMUL", "1"))
            LPV = int(os.environ.get("K_LPV", "1"))
            LEV = int(os.environ.get("K_LEV", "1"))
            for g in range(NGROUPS):
                ensure_chunk(CHUNK_OF_GROUP[min(g + 2, NGROUPS - 1)])
                for hh in range(2):
                    ep, em = emit_qk_exp(g, hh,
                                         split_exp=(g == 0 and hh == 0))
                    step_evac(min(CAP, max(0, len(unev) - LEV)))
                    step_pv(min(CAP, max(0, len(unpv) - LPV)))
                    step_emul(min(CAP, max(0, len(unmul) - LMUL)))
                    unmul.append((g, hh, ep, em))
            # drain the tail oldest-first; the final store is split so its
            # first half overlaps the last PV
            while unmul or unpv or unev:
                step_emul(2)
                step_pv(2)
                last = len(unpv) == 0 and len(unmul) == 0 and len(unev) == 1
                if unev:
                    g_, hh_, o_ = unev.pop(0)
                    emit_evac_store(g_, hh_, o_, split=last)
                if unev:
                    g_, hh_, o_ = unev.pop(0)
                    emit_evac_store(g_, hh_, o_)

    nc.compile()
    _BUILD_CACHE["nc"] = nc
    return nc


def _pos_mlp(inputs):
    """Host-side dynamic-pos-bias MLP + Toeplitz index (f64)."""
    f64 = lambda name: np.asarray(inputs[name], np.float64)

    def layernorm(x, g, b):
        m = x.mean(-1, keepdims=True)
        v = ((x - m) ** 2).mean(-1, keepdims=True)
        return (x - m) / np.sqrt(v + 1e-5) * g + b

    h = w = 16
    bh, bw = np.meshgrid(np.arange(1 - h, h), np.arange(1 - w, w),
                         indexing="ij")
    biases = np.stack([bh, bw], -1).reshape(-1, 2).astype(np.float64)
    pos = biases @ f64("w_proj") + f64("b_proj")
    for i in (1, 2, 3):
        pos = np.maximum(
            layernorm(pos, f64(f"ln{i}_g"), f64(f"ln{i}_b")), 0.0)
        pos = pos @ f64(f"w{i}") + f64(f"b{i}")
    ch, cw = np.meshgrid(np.arange(h), np.arange(w), indexing="ij")
    coords = np.stack([ch.reshape(-1), cw.reshape(-1)])
    rel = coords[:, :, None] - coords[:, None, :]
    idx = (rel[0] + h - 1) * (2 * w - 1) + (rel[1] + w - 1)   # [n, m]
    return pos, idx


def build_in_maps(inputs):
    q = np.asarray(inputs["q"], np.float32)
    k = np.asarray(inputs["k"], np.float32)
    v = np.asarray(inputs["v"], np.float32)
    hh = int(np.asarray(inputs["h"]))
    ww = int(np.asarray(inputs["w"]))
    assert hh == 16 and ww == 16, (hh, ww)

    pos, idx = _pos_mlp(inputs)
    # bias_z[n, m, H] in z units with the -6 octave shift baked in
    bias_z = pos[idx] * W3_SCALE + BSHIFT

    def q_layout(x):
        # [128 w, 256 n, 32 d] -> [128 p=(bi,d), (g, hh, n)] fp16, *ALPHA
        x5 = (x * np.float32(ALPHA)).reshape(16, 2, 4, 256, 32)
        return np.ascontiguousarray(
            x5.transpose(2, 4, 0, 1, 3).reshape(128, 8192).astype(np.float16))

    def k_layout(x):
        # [128 w, 256 m, 32 d] -> [128 p=(bi,d), (g, hh, mb, m)] fp16
        x6 = x.reshape(16, 2, 4, 2, 128, 32)        # g hh bi mb m d
        return np.ascontiguousarray(
            x6.transpose(2, 5, 0, 1, 3, 4).reshape(128, 8192)
            .astype(np.float16))

    def v_layout(x):
        # [128 p=m, (b 128, c 2, e 33)] fp16; e==32 -> 1.0
        v4 = x.reshape(128, 2, 128, 32)             # b c p e
        out = np.ones((128, 128, 2, 33), np.float32)
        out[:, :, :, :32] = v4.transpose(2, 0, 1, 3)
        return np.ascontiguousarray(out.reshape(128, 8448).astype(np.float16))

    ident = np.eye(128, dtype=np.float32)
    in_maps = []
    for c in range(NCORES):
        bz = bias_z[:, :, c]                        # [n, m]
        # bfold[mp, (mb, n)] = bias_z[n, mb*128+mp]
        bf = np.empty((128, 2, 256), np.float32)
        for mb in range(2):
            bf[:, mb, :] = bz[:, 128 * mb:128 * mb + 128].T
        bfold = np.ascontiguousarray(bf.reshape(128, 512))
        expbd = np.ascontiguousarray(
            np.exp2(bfold.astype(np.float64) / 1024.0).astype(np.float16))
        in_maps.append({
            "qd": q_layout(q[:, c]),
            "kd": k_layout(k[:, c]),
            "vd": v_layout(v[:, c]),
            "bfold": bfold,
            "expbd": expbd,
            "identd": ident,
        })
    return in_maps


def unshard_out(raw):
    # raw [128 p, (g 16, hh 2, j 8, e 33)] fp16 -> [B, N, D] f32 (normalize)
    r5 = raw.reshape(128, 16, 2, 8, 33).astype(np.float32)  # p g hh j e
    O = r5[..., :32]
    Z = r5[..., 32]
    out = O / Z[..., None]
    # b = (g*2+hh)*4 + j//2 ; n = (j%2)*128 + p ; d = e
    o6 = out.reshape(128, 16, 2, 4, 2, 32)           # p g hh bi nb e
    return np.ascontiguousarray(
        o6.transpose(1, 2, 3, 4, 0, 5).reshape(128, 256, 32))


def kernel(**inputs):
    from concourse.bass_utils import run_bass_kernel_spmd

    nc = _build()
    in_maps = build_in_maps(inputs)
    res = run_bass_kernel_spmd(nc, in_maps, core_ids=list(range(NCORES)))
    out = np.empty((B, H, N, D), np.float32)
    for c in range(NCORES):
        out[:, c] = unshard_out(res.results[c]["out"])
    return out
